# revision 40
# baseline (speedup 1.0000x reference)
"""GCN3 (3-layer graph conv + log_softmax) Trainium2 Bass kernel, 8-way SPMD.

Sharding: nodes row-sharded across 8 cores (12500 rows each); edges co-located
with their destination-row partition (host-sorted by dest tile). The [N,128]
pre-activation matrix is AllGathered in TWO chunks per layer (local row ranges
[0,6400) and [6400,12500) of every core), issued eagerly as soon as the
producing tiles finish, so the collective overlaps compute. Each layer's edge
work is split into phase A (source sub-tables 0,1 = AG chunk 0) and phase B
(sub-tables 2,3 = AG chunk 1): phase A runs under AG chunk 1's shadow, with
per-tile partial sums spilled from PSUM to SBUF between phases. Edge messages
are fetched with dma_gather (256B bf16 rows, int16 indices over 4 sub-tables);
segment-sum is S.T @ G per 128-edge chunk on the tensor engine, where
S[j,d] = vals[j] * (rows_local[j] == d) is built either in one DVE
tensor_scalar op or (for ~40% of chunks, load-balancing onto the idle ACT
engine) as Relu(vals * (1 - |iota - rows_local|)) in two activation ops.
Tails are deferred one group so in-order engines never stall behind PSUM
waits; phase-A partials fold back into phase-B PSUM via an identity matmul;
LN-affine + ReLU fuse into a single scale/bias activation.
"""

import numpy as np

P = 128


class _Cfg:
    def __init__(self, N, NFEAT, NHID, NCLASS, CORES, EPS=1e-5,
                 GROUP_TILES=3, ROW_WINDOW=1024, NQUEUES=4, CAP_CH=8,
                 PSG=6, PTR=1, PPR=1, GBUFS=4, TBUFS=3, MBUFS=3,
                 SCRATCH=16384):
        import os as _os
        GROUP_TILES = int(_os.environ.get("GCN_GT", GROUP_TILES))
        NQUEUES = int(_os.environ.get("GCN_NQ", NQUEUES))
        CAP_CH = int(_os.environ.get("GCN_CAP", CAP_CH))
        PSG = int(_os.environ.get("GCN_PSG", PSG))
        GBUFS = int(_os.environ.get("GCN_GBUFS", GBUFS))
        MBUFS = int(_os.environ.get("GCN_MBUFS", MBUFS))
        TBUFS = int(_os.environ.get("GCN_TBUFS", TBUFS))
        SCRATCH = int(_os.environ.get("GCN_SCRATCH", SCRATCH))
        self.SCRATCH = SCRATCH
        self.BF16 = _os.environ.get("GCN_BF16", "1") == "1"
        self.SBUFS = int(_os.environ.get("GCN_SBUFS", 12))
        self.DEFER = _os.environ.get("GCN_DEFER", "1") == "1"
        self.ACTS = int(_os.environ.get("GCN_ACTS", "30"))  # % S-builds on ACT
        self.POOLS = int(_os.environ.get("GCN_POOLS", "0"))  # % on Pool/gpsimd
        self.N, self.NFEAT, self.NHID, self.NCLASS = N, NFEAT, NHID, NCLASS
        self.CORES, self.EPS = CORES, EPS
        assert N % CORES == 0
        self.NLOC = N // CORES
        self.TILES = -(-self.NLOC // P)
        self.tile_rows = [min(P, self.NLOC - t * P) for t in range(self.TILES)]
        self.KCH = -(-NFEAT // P)          # k chunks for x @ W1
        self.KPAD = self.KCH * P
        # ---- AG chunking: 2 chunks by local tile range
        t0 = self.TILES // 2 + 1           # 50 tiles -> rows 0..6400
        self.CH_TILES = [t0, self.TILES - t0]
        self.CH_TOFF = [0, t0]
        self.CH_ROWS = [t0 * P, self.NLOC - t0 * P]
        self.CH_ROFF = [0, t0 * P]
        self.NSUB = 4
        # sub-table s: chunk k = s//2, cores (s%2)*CORES/2 .. +CORES/2
        self.HALF = CORES // 2
        self.sub_rows = [self.HALF * self.CH_ROWS[s // 2] for s in range(4)]
        assert max(self.sub_rows) <= 32768
        self.phase_s = [(0, 1), (2, 3)]
        self.GROUP_TILES = GROUP_TILES
        # groups chunk-aligned
        self.groups = []
        for k in range(2):
            lo, n = self.CH_TOFF[k], self.CH_TILES[k]
            for g in range(lo, lo + n, GROUP_TILES):
                self.groups.append(list(range(g, min(g + GROUP_TILES, lo + n))))
        # last group index whose tiles are all < CH_TOFF[1]
        self.k0_last_group = max(i for i, g in enumerate(self.groups)
                                 if g[-1] < self.CH_TOFF[1])
        self.ROW_WINDOW = ROW_WINDOW
        self.NQUEUES = NQUEUES
        self.CAP_CH = CAP_CH
        self.PSG, self.PTR, self.PPR = PSG, PTR, PPR
        self.GBUFS, self.TBUFS, self.MBUFS = GBUFS, TBUFS, MBUFS

    def sub_of(self, cc):
        """cc: global col array -> (s, idx_in_subtable)."""
        c = cc // self.NLOC
        r0 = cc % self.NLOC
        k = (r0 >= self.CH_ROWS[0]).astype(np.int64)
        half = (c >= self.HALF).astype(np.int64)
        s = 2 * k + half
        rng = np.where(k == 0, self.CH_ROWS[0], self.CH_ROWS[1])
        roff = np.where(k == 0, 0, self.CH_ROFF[1])
        idx = (c % self.HALF) * rng + (r0 - roff)
        return s, idx


def _balance_perm(cfg, rows, cols, heavy_frac=None):
    """Per-core node relabeling that reshapes (tile, sub-table) edge-count
    buckets bimodally (most ~<=480 -> 4 chunks, some ~<=610 -> 5) instead of
    all ~510 -> ceil 5. Permutes only within each (core, AG-chunk) row range,
    so every edge's sub-table classification is unchanged.

    Returns pi with: new node i corresponds to original node pi[i].
    """
    import os as _os
    if heavy_frac is None:
        heavy_frac = float(_os.environ.get("GCN_HF", "0.45"))
    N, NLOC, L = cfg.N, cfg.NLOC, rows.shape[0]
    greedy = _os.environ.get("GCN_DEAL", "greedy") == "greedy"
    d12 = np.zeros((N, L * cfg.NSUB), np.float64)
    for l in range(L):
        s_l, _ = cfg.sub_of(cols[l])
        np.add.at(d12, (rows[l], l * cfg.NSUB + s_l), 1.0)
    d_tot = d12.sum(1)
    pi = np.empty(N, np.int64)
    for c in range(cfg.CORES):
        base = c * NLOC
        for k in range(2):
            lo = base + cfg.CH_ROFF[k]
            n = cfg.CH_ROWS[k]
            ids = np.arange(lo, lo + n)
            order = ids[np.argsort(d_tot[ids], kind="stable")]
            tiles = list(range(cfg.CH_TOFF[k],
                               cfg.CH_TOFF[k] + cfg.CH_TILES[k]))
            caps = [cfg.tile_rows[t] for t in tiles]
            nheavy = max(1, int(round(len(tiles) * heavy_frac)))
            light_t, heavy_t = tiles[:-nheavy], tiles[-nheavy:]
            ncap_h = sum(cfg.tile_rows[t] for t in heavy_t)
            light_rows, heavy_rows = order[:n - ncap_h], order[n - ncap_h:]
            for cls_tiles, cls_rows in ((light_t, light_rows),
                                        (heavy_t, heavy_rows)):
                fill = {t: 0 for t in cls_tiles}
                if not greedy:
                    # stratified round-robin deal of degree-sorted rows
                    slot_seq = []
                    mx = max(cfg.tile_rows[t] for t in cls_tiles)
                    for j in range(mx):
                        for t in cls_tiles:
                            if j < cfg.tile_rows[t]:
                                slot_seq.append(t)
                    for rid, t in zip(cls_rows, slot_seq):
                        pi[base + t * P + fill[t]] = rid
                        fill[t] += 1
                    continue
                # load-feedback deal: per round, pair heaviest remaining
                # rows with the per-dim least-loaded tiles
                ct = np.array(cls_tiles)
                caps = np.array([cfg.tile_rows[t] for t in ct])
                target = d12[cls_rows].sum(0) / caps.sum()  # per-slot mean
                loads = np.zeros((len(ct), d12.shape[1]))
                nfill = np.zeros(len(ct), np.int64)
                rows_desc = cls_rows[::-1]
                pos = 0
                while pos < len(rows_desc):
                    avail = np.nonzero(nfill < caps)[0]
                    m = min(len(avail), len(rows_desc) - pos)
                    batch = rows_desc[pos:pos + m]
                    # relative max-dim load, minus slots filled so far
                    score = (loads[avail] / np.maximum(target, 1e-9)
                             ).max(1) - nfill[avail]
                    order_t = avail[np.argsort(score, kind="stable")]
                    sel = order_t[:m]
                    for rid, ti in zip(batch, sel):
                        t = int(ct[ti])
                        pi[base + t * P + nfill[ti]] = rid
                        loads[ti] += d12[rid]
                        nfill[ti] += 1
                    pos += m
    return pi


class _Plan:
    """Static (data-dependent but compile-time) structure shared by all cores."""
    def __init__(self, cfg, rows, cols):
        self.cfg = cfg
        L = rows.shape[0]
        self.L = L
        self.nch = np.zeros((L, cfg.TILES, cfg.NSUB), dtype=np.int64)
        for l in range(L):
            t_of = (rows[l] % cfg.NLOC) // P
            s_of, _ = cfg.sub_of(cols[l])
            core_of = rows[l] // cfg.NLOC
            key = (core_of * cfg.TILES + t_of) * cfg.NSUB + s_of
            cnt = np.bincount(key, minlength=cfg.CORES * cfg.TILES * cfg.NSUB)
            cnt = cnt.reshape(cfg.CORES, cfg.TILES, cfg.NSUB)
            self.nch[l] = -(-cnt.max(axis=0) // P)
        # call list per layer: (p, g_idx, s, [(t, nch), ...], ch_off), ordered
        # (phase, group, s). One dma_gather call stays <= CAP_CH chunks.
        self.CAP_CH = cfg.CAP_CH
        self.calls = []
        self.tot_ch = []
        for l in range(L):
            calls_l = []
            ch_off = 0
            for p in range(2):
                for gi, g in enumerate(cfg.groups):
                    for s in cfg.phase_s[p]:
                        pieces = []
                        acc = 0
                        for t in g:
                            left = int(self.nch[l, t, s])
                            while left > 0:
                                take = min(left, self.CAP_CH - acc)
                                pieces.append((t, take))
                                acc += take
                                left -= take
                                if acc == self.CAP_CH:
                                    calls_l.append((p, gi, s, pieces, ch_off))
                                    ch_off += acc
                                    pieces, acc = [], 0
                        if acc > 0:
                            calls_l.append((p, gi, s, pieces, ch_off))
                            ch_off += acc
            self.calls.append(calls_l)
            self.tot_ch.append(ch_off)
        self.max_call_ch = max(
            max((sum(c for _, c in tl) for _, _, _, tl, _ in cl), default=1)
            for cl in self.calls)
        # (p, group) -> (chunk range, call list) for metadata loads
        self.group_spans = []
        for l in range(L):
            spans = {}
            for ci, (p, gi, s, tl, ch_off) in enumerate(self.calls[l]):
                n = sum(c for _, c in tl)
                lo, hi, cs = spans.get((p, gi), (ch_off, ch_off + n, []))
                spans[(p, gi)] = (min(lo, ch_off), max(hi, ch_off + n), cs + [ci])
            self.group_spans.append(spans)


def _preprocess(cfg, plan, x, rows, cols, vals, W1):
    """Build per-core input maps. All heavy lifting is layout/permutation."""
    L = plan.L
    # --- x shard, transposed + k-padded: [KPAD, NLOC] per core
    xT = np.zeros((cfg.CORES, cfg.KPAD, cfg.NLOC), dtype=np.float32)
    for c in range(cfg.CORES):
        xT[c, :cfg.NFEAT, :] = x[c * cfg.NLOC:(c + 1) * cfg.NLOC, :].T
    W1p = np.zeros((cfg.KPAD, cfg.NHID), dtype=np.float32)
    W1p[:cfg.NFEAT] = W1

    # --- edge data per (core, layer), ordered by call layout
    idx_arr = [[None] * L for _ in range(cfg.CORES)]
    row_arr = [[None] * L for _ in range(cfg.CORES)]
    val_arr = [[None] * L for _ in range(cfg.CORES)]
    for l in range(L):
        core_of = rows[l] // cfg.NLOC
        for c in range(cfg.CORES):
            m = core_of == c
            r = rows[l][m] - c * cfg.NLOC
            cc = cols[l][m]
            vv = vals[l][m]
            t_of = r // P
            s_of, sidx = cfg.sub_of(cc)
            order = np.argsort(t_of * cfg.NSUB + s_of, kind="stable")
            r, sidx, vv = r[order], sidx[order], vv[order]
            key = t_of[order] * cfg.NSUB + s_of[order]
            cnt = np.bincount(key, minlength=cfg.TILES * cfg.NSUB)
            off = np.zeros(cfg.TILES * cfg.NSUB + 1, dtype=np.int64)
            np.cumsum(cnt, out=off[1:])
            tot = plan.tot_ch[l] * P
            gidx = np.zeros(tot, dtype=np.int16)
            rloc = np.zeros(tot, dtype=np.float32)
            vpad = np.zeros(tot, dtype=np.float32)
            used = np.zeros(cfg.TILES * cfg.NSUB, dtype=np.int64)
            for p, gi, s, tl, ch_off in plan.calls[l]:
                o = ch_off * P
                for t, nch in tl:
                    k = t * cfg.NSUB + s
                    # (t,s) runs may span calls: continue from cursor
                    n = min(int(cnt[k]) - int(used[k]), nch * P)
                    if n > 0:
                        st = int(off[k]) + int(used[k])
                        sl = slice(st, st + n)
                        gidx[o:o + n] = sidx[sl].astype(np.int16)
                        rloc[o:o + n] = (r[sl] - t * P).astype(np.float32)
                        vpad[o:o + n] = vv[sl]
                        used[k] += n
                    # pad slots keep gidx=0 (valid row of sub-table), val=0
                    o += nch * P
            # wrap indices: slot i -> [i%16, i//16], replicated to 128 parts
            w16 = gidx.reshape(-1, 16).T
            idx_arr[c][l] = np.tile(w16, (8, 1)).copy()
            row_arr[c][l] = rloc.reshape(-1, P).T.copy()
            val_arr[c][l] = vpad.reshape(-1, P).T.copy()
    return xT, W1p, idx_arr, row_arr, val_arr


def _build_program(cfg, plan, apply_b, apply_g, apply_lnb, apply_bout):
    import os as _os
    SKIP_SEG = _os.environ.get("GCN_SKIP_SEG", "0") == "1"
    SKIP_TAIL = _os.environ.get("GCN_SKIP_TAIL", "0") == "1"
    SKIP_AG = _os.environ.get("GCN_SKIP_AG", "0") == "1"
    SKIP_DENSE = _os.environ.get("GCN_SKIP_DENSE", "0") == "1"
    import concourse.bacc as bacc
    import concourse.tile as tile
    import concourse.mybir as mybir
    from concourse import library_config

    f32 = mybir.dt.float32
    i16 = mybir.dt.int16
    hdt = mybir.dt.bfloat16 if cfg.BF16 else f32
    Alu = mybir.AluOpType
    Act = mybir.ActivationFunctionType
    L, NH, NC = plan.L, cfg.NHID, cfg.NCLASS

    nc = bacc.Bacc("TRN2", target_bir_lowering=False, debug=False,
                   enable_asserts=False, num_devices=cfg.CORES,
                   num_swdge_queues=cfg.NQUEUES,
                   dynamic_dma_scratch_size=cfg.SCRATCH)

    # ---- I/O tensors
    xT_d = nc.dram_tensor("xT", [cfg.KPAD, cfg.NLOC], hdt,
                          kind="ExternalInput")
    W1_d = nc.dram_tensor("W1p", [cfg.KPAD, NH], hdt, kind="ExternalInput")
    W2_d = nc.dram_tensor("W2", [NH, NH], f32, kind="ExternalInput")
    W3_d = nc.dram_tensor("W3", [NH, NH], f32, kind="ExternalInput")
    Wo_d = nc.dram_tensor("Wout", [NH, NC], f32, kind="ExternalInput")
    b_d = [nc.dram_tensor(f"b{l+1}", [P, NH], f32, kind="ExternalInput")
           for l in range(L)]
    bo_d = nc.dram_tensor("bout", [P, NC], f32, kind="ExternalInput")
    g_d = nc.dram_tensor("lng", [P, NH], f32, kind="ExternalInput")
    lb_d = nc.dram_tensor("lnb", [P, NH], f32, kind="ExternalInput")
    idx_d = [nc.dram_tensor(f"idx{l}", [P, plan.tot_ch[l] * 8], i16,
                            kind="ExternalInput") for l in range(L)]
    met_d = [nc.dram_tensor(f"meta{l}", [P, plan.tot_ch[l] * 4], f32,
                            kind="ExternalInput") for l in range(L)]
    ib_d = nc.dram_tensor("identb", [P, P], hdt, kind="ExternalInput")
    cst_d = nc.dram_tensor("consts", [P, 2 * P], f32, kind="ExternalInput")
    out_d = nc.dram_tensor("out", [cfg.NLOC, NC], f32, kind="ExternalOutput")

    W_next = [W2_d, W3_d]  # weight applied in tail of layers 0,1

    with tile.TileContext(nc) as tc:
        with tc.tile_pool(name="sb", bufs=1) as sb, \
             tc.tile_pool(name="sbG", bufs=cfg.GBUFS) as sbG, \
             tc.tile_pool(name="sbM", bufs=cfg.MBUFS) as sbM, \
             tc.tile_pool(name="sbX", bufs=2) as sbX, \
             tc.tile_pool(name="sbP", bufs=1) as sbP, \
             tc.tile_pool(name="sbS", bufs=cfg.SBUFS) as sbS, \
             tc.tile_pool(name="sbT", bufs=cfg.TBUFS) as sbT, \
             tc.tile_pool(name="psg", bufs=cfg.PSG, space="PSUM") as psg, \
             tc.tile_pool(name="pst", bufs=1, space="PSUM") as pst, \
             tc.tile_pool(name="dram", bufs=1, space="DRAM") as dram:

            nc.gpsimd.load_library(library_config.mlp)

            # ---- constants (iota row + identity, host-provided)
            cst_t = sb.tile([P, 2 * P], f32)
            nc.sync.dma_start(cst_t[:], cst_d[:])
            iota_t = cst_t[:, 0:P]
            ident = cst_t[:, P:2 * P]
            eps_t = sb.tile([P, 1], f32)
            nc.vector.memset(eps_t[:], cfg.EPS)
            W1_t = sb.tile([P, cfg.KCH * NH], hdt)
            nc.sync.dma_start(
                W1_t[:].rearrange("p (k f) -> p k f", k=cfg.KCH),
                W1_d[:].rearrange("(k p) f -> p k f", p=P))
            Wn_t = []
            for l in range(L - 1):
                w = sb.tile([P, NH], f32, tag=f"wn{l}", name=f"wn{l}")
                nc.sync.dma_start(w[:], W_next[l][:])
                Wn_t.append(w)
            Wo_t = sb.tile([P, NC], f32)
            nc.sync.dma_start(Wo_t[:], Wo_d[:])
            ib_t = sb.tile([P, P], hdt)
            nc.sync.dma_start(ib_t[:], ib_d[:])
            b_t = []
            for l in range(L):
                if apply_b[l]:
                    t_ = sb.tile([P, NH], f32, tag=f"b{l}", name=f"bt{l}")
                    nc.sync.dma_start(t_[:], b_d[l][:])
                    b_t.append(t_)
                else:
                    b_t.append(None)
            g_t = lb_t = bo_t = None
            if apply_g:
                g_t = sb.tile([P, NH], f32)
                nc.sync.dma_start(g_t[:], g_d[:])
            if apply_lnb:
                lb_t = sb.tile([P, NH], f32)
                nc.sync.dma_start(lb_t[:], lb_d[:])
            if apply_bout:
                bo_t = sb.tile([P, NC], f32)
                nc.sync.dma_start(bo_t[:], bo_d[:])

            # ---- DRAM intermediates (per layer, per AG chunk)
            ag_in = [[dram.tile([cfg.CH_ROWS[k], NH], hdt, tag=f"agin{l}k{k}",
                                name=f"agin{l}k{k}") for k in range(2)]
                     for l in range(L)]
            aspace = "Shared" if cfg.CORES > 4 else "Local"
            pre_ch = [[dram.tile([cfg.CORES * cfg.CH_ROWS[k], NH], hdt,
                                 tag=f"pref{l}k{k}", addr_space=aspace,
                                 name=f"pref{l}k{k}") for k in range(2)]
                      for l in range(L)]
            rg = [list(range(cfg.CORES))]

            def issue_ag(l, k):
                if SKIP_AG:
                    return
                if cfg.CORES == 1:
                    nc.sync.dma_start(pre_ch[l][k][:, :], ag_in[l][k][:, :])
                else:
                    nc.gpsimd.collective_compute(
                        "AllGather", mybir.AluOpType.bypass,
                        ins=[ag_in[l][k].opt()], outs=[pre_ch[l][k].opt()],
                        replica_groups=rg)

            def agin_write(l, t, tl_, src):
                k = 0 if t < cfg.CH_TOFF[1] else 1
                r0 = t * P - cfg.CH_ROFF[k]
                nc.sync.dma_start(ag_in[l][k][r0:r0 + tl_, :], src)

            # ---- phase 0: pre1 = x @ W1 (row-windowed)
            ag0_issued = False
            for w0 in ([] if SKIP_DENSE else
                       range(0, cfg.NLOC, cfg.ROW_WINDOW)):
                wl = min(cfg.ROW_WINDOW, cfg.NLOC - w0)
                xsl = sbX.tile([P, cfg.KCH * cfg.ROW_WINDOW], hdt, tag="xsl")
                x3 = xsl[:].rearrange("p (k r) -> p k r", k=cfg.KCH)
                nc.sync.dma_start(
                    x3[:, :, :wl],
                    xT_d[:].rearrange("(k p) r -> p k r", p=P)[:, :, w0:w0 + wl])
                for t0 in range(0, wl, P):
                    t = (w0 + t0) // P
                    tl_ = cfg.tile_rows[t]
                    ps = psg.tile([P, NH], f32, space="PSUM", tag="ps")
                    for k in range(cfg.KCH):
                        nc.tensor.matmul(
                            ps[:tl_], lhsT=x3[:, k, t0:t0 + tl_],
                            rhs=W1_t[:, k * NH:(k + 1) * NH],
                            start=(k == 0), stop=(k == cfg.KCH - 1))
                    pre_sb = sbT.tile([P, NH], hdt, tag="pre")
                    nc.scalar.copy(pre_sb[:tl_], ps[:tl_])
                    agin_write(0, t, tl_, pre_sb[:tl_])
                if not ag0_issued and w0 + wl >= cfg.CH_ROWS[0]:
                    issue_ag(0, 0)
                    ag0_issued = True
            if not SKIP_DENSE:
                issue_ag(0, 1)

            nidx_regs = {}
            # ---- layers
            for l in range(L):
                spans = plan.group_spans[l]
                # per-(tile, phase) chunk bookkeeping for start/stop flags
                tile_total = {(t, p): int(plan.nch[l, t, cfg.phase_s[p]].sum())
                              for t in range(cfg.TILES) for p in range(2)}
                tile_seen = {}
                psum_of = {}
                part_of = {}
                part_merged = set()
                sctr = 0

                def spill_A(gi, g):
                    # spill phase-A partials PSUM -> SBUF (bf16)
                    if SKIP_SEG:
                        return
                    for t in g:
                        if tile_total[(t, 0)] == 0:
                            continue
                        pa = sbP.tile([P, NH], hdt, tag=f"pt{t}",
                                      name=f"part{t}")
                        nc.scalar.copy(pa[:], psum_of.pop((t, 0))[:])
                        part_of[t] = pa

                def tails_B(gi, g):
                    if SKIP_TAIL:
                        if l == L - 1:
                            for t in g:
                                tl_ = cfg.tile_rows[t]
                                res0 = sbT.tile([P, NC], f32, tag="res")
                                nc.vector.memset(res0[:], 0.0)
                                nc.sync.dma_start(
                                    out_d[t * P:t * P + tl_, :], res0[:tl_])
                        return
                    lgm_g = None
                    for t in g:
                        tl_ = cfg.tile_rows[t]
                        pb = (psum_of.pop((t, 1), None)
                              if not SKIP_SEG else None)
                        pa = part_of.pop(t, None)
                        if t in part_merged:
                            pa = None
                        if pb is not None and pa is not None:
                            h_t = sbT.tile([P, NH], f32, tag="h2")
                            nc.vector.tensor_tensor(out=h_t[:], in0=pb[:],
                                                    in1=pa[:], op=Alu.add)
                        elif pb is not None:
                            h_t = pb
                        elif pa is not None:
                            h_t = pa
                        else:
                            h_t = sbT.tile([P, NH], f32, tag="h2")
                            nc.vector.memset(h_t[:], 0.0)
                        if b_t[l] is not None:
                            h2 = sbT.tile([P, NH], f32, tag="h3")
                            nc.vector.tensor_tensor(out=h2[:], in0=h_t[:],
                                                    in1=b_t[l][:],
                                                    op=Alu.add)
                            h_t = h2
                        stats6 = sbT.tile([P, 6], f32, tag="st6")
                        nc.vector.bn_stats(stats6[:], h_t[:])
                        stats2 = sbT.tile([P, 2], f32, tag="st2")
                        nc.vector.bn_aggr(stats2[:], stats6[:])
                        std_t = sbT.tile([P, 1], f32, tag="std")
                        nc.scalar.activation(std_t[:], stats2[:, 1:2],
                                             Act.Sqrt, bias=eps_t[:, 0:1])
                        inv_t = sbT.tile([P, 1], f32, tag="inv")
                        nc.vector.reciprocal(inv_t[:], std_t[:])
                        nms = sbT.tile([P, 1], f32, tag="nms")
                        nc.vector.tensor_scalar(
                            out=nms[:], in0=stats2[:, 0:1],
                            scalar1=inv_t[:, 0:1], scalar2=-1.0,
                            op0=Alu.mult, op1=Alu.mult)
                        if g_t is None and lb_t is None:
                            # fused LN-affine + ReLU on ACT
                            hr = sbT.tile([P, NH], f32, tag="hr")
                            nc.scalar.activation(hr[:], h_t[:], Act.Relu,
                                                 bias=nms[:, 0:1],
                                                 scale=inv_t[:, 0:1])
                        else:
                            t_n = sbT.tile([P, NH], f32, tag="tn")
                            nc.vector.tensor_scalar(
                                out=t_n[:], in0=h_t[:],
                                scalar1=inv_t[:, 0:1],
                                scalar2=nms[:, 0:1], op0=Alu.mult,
                                op1=Alu.add)
                            if g_t is not None:
                                t_g = sbT.tile([P, NH], f32, tag="tg")
                                nc.vector.tensor_tensor(out=t_g[:],
                                                        in0=t_n[:],
                                                        in1=g_t[:],
                                                        op=Alu.mult)
                                t_n = t_g
                            if lb_t is not None:
                                t_b = sbT.tile([P, NH], f32, tag="tb")
                                nc.vector.tensor_tensor(out=t_b[:],
                                                        in0=t_n[:],
                                                        in1=lb_t[:],
                                                        op=Alu.add)
                                t_n = t_b
                            hr = sbT.tile([P, NH], f32, tag="hr")
                            nc.scalar.activation(hr[:], t_n[:], Act.Relu)
                        # transpose
                        hT_ps = pst.tile([P, P], f32, space="PSUM",
                                         tag="ptr", bufs=cfg.PTR)
                        nc.tensor.transpose(out=hT_ps[:], in_=hr[:],
                                            identity=ident)
                        hT = sbT.tile([P, P], f32, tag="hT")
                        nc.vector.tensor_copy(hT[:], hT_ps[:])
                        if l < L - 1:
                            pr_ps = pst.tile([P, NH], f32, space="PSUM",
                                             tag="ppr", bufs=cfg.PPR)
                            nc.tensor.matmul(pr_ps[:tl_], lhsT=hT[:, :tl_],
                                             rhs=Wn_t[l][:],
                                             start=True, stop=True)
                            pre_sb = sbT.tile([P, NH], hdt, tag="pre")
                            nc.scalar.copy(pre_sb[:tl_], pr_ps[:tl_])
                            agin_write(l + 1, t, tl_, pre_sb[:tl_])
                        else:
                            lg_ps = pst.tile([P, NC], f32, space="PSUM",
                                             tag="ppr", bufs=cfg.PPR)
                            nc.tensor.matmul(lg_ps[:tl_], lhsT=hT[:, :tl_],
                                             rhs=Wo_t[:], start=True,
                                             stop=True)
                            ti = t - g[0]
                            if ti == 0:
                                lgm_g = sbT.tile([P, len(g) * NC], f32,
                                                 tag="lgm", name=f"lgm{gi}")
                                if any(cfg.tile_rows[tt] < P for tt in g):
                                    nc.vector.memset(lgm_g[:], 0.0)
                            if bo_t is not None:
                                lg = sbT.tile([P, NC], f32, tag="lg")
                                nc.vector.tensor_tensor(
                                    out=lg[:tl_], in0=lg_ps[:tl_],
                                    in1=bo_t[:tl_], op=Alu.add)
                                lg_v = lg[:tl_]
                            else:
                                lg_v = lg_ps[:tl_]
                            mx = sbT.tile([P, 1], f32, tag="mx",
                                          name=f"mx{t}")
                            nc.vector.reduce_max(mx[:tl_], lg_v,
                                                 axis=mybir.AxisListType.X)
                            nc.vector.tensor_scalar(
                                out=lgm_g[:tl_, ti * NC:(ti + 1) * NC],
                                in0=lg_v, scalar1=mx[:tl_, 0:1],
                                scalar2=None, op0=Alu.subtract)
                    # batched log_softmax epilogue for the group
                    if l == L - 1:
                        ng = len(g)
                        ex_g = sbT.tile([P, ng * NC], f32, tag="exg",
                                        name=f"exg{gi}")
                        nc.scalar.activation(ex_g[:], lgm_g[:], Act.Exp)
                        se_g = sbT.tile([P, ng], f32, tag="seg",
                                        name=f"seg{gi}")
                        nc.vector.reduce_sum(
                            se_g[:].rearrange("p g -> p g ()"),
                            ex_g[:].rearrange("p (g c) -> p g c", c=NC),
                            axis=mybir.AxisListType.X)
                        lse_g = sbT.tile([P, ng], f32, tag="lseg",
                                         name=f"lseg{gi}")
                        nc.scalar.activation(lse_g[:], se_g[:], Act.Ln)
                        for t in g:
                            tl_ = cfg.tile_rows[t]
                            ti = t - g[0]
                            res = sbT.tile([P, NC], f32, tag="res")
                            nc.vector.tensor_scalar(
                                out=res[:tl_],
                                in0=lgm_g[:tl_, ti * NC:(ti + 1) * NC],
                                scalar1=lse_g[:tl_, ti:ti + 1],
                                scalar2=None, op0=Alu.subtract)
                            nc.sync.dma_start(
                                out_d[t * P:t * P + tl_, :], res[:tl_])

                def finish_group(p, gi, g):
                    if p == 0:
                        spill_A(gi, g)
                    else:
                        tails_B(gi, g)
                        # eager AG of next layer's chunk 0 once all
                        # chunk-0 tiles' tails have run
                        if l < L - 1 and gi == cfg.k0_last_group:
                            issue_ag(l + 1, 0)

                for p in range(2):
                    pending = None
                    for gi, g in enumerate(cfg.groups):
                        lo, hi, cis = spans.get((p, gi), (0, 0, []))
                        gch = hi - lo
                        if gch > 0:
                            meta_t = sbM.tile([P, gch * 4], f32, tag="meta")
                            nc.sync.dma_start(meta_t[:],
                                              met_d[l][:, lo * 4:hi * 4])
                            rows_t = meta_t[:, 0:gch]
                            vals_t = meta_t[:, gch:2 * gch]
                            rneg_t = meta_t[:, 2 * gch:3 * gch]
                            vneg_t = meta_t[:, 3 * gch:4 * gch]
                            idx_t = sbM.tile([P, gch * 8], i16, tag="idx")
                            nc.sync.dma_start(idx_t[:],
                                              idx_d[l][:, lo * 8:hi * 8])

                        for ci in cis:
                            _, _, s, tl_list, ch_off = plan.calls[l][ci]
                            ncall = sum(c for _, c in tl_list)
                            nidx = ncall * P
                            G = sbG.tile([P, plan.CAP_CH * NH], hdt, tag="G")
                            k = s // 2
                            sub0 = (s % 2) * cfg.HALF * cfg.CH_ROWS[k]
                            sub_rows = cfg.sub_rows[s]
                            if nidx not in nidx_regs:
                                nidx_regs[nidx] = nc.gpsimd.to_reg(nidx)
                            nc.gpsimd.dma_gather(
                                G[:].rearrange("p (c f) -> p c f", f=NH)[:, :ncall, :],
                                pre_ch[l][k][sub0:sub0 + sub_rows, :],
                                idx_t[:, (ch_off - lo) * 8:(ch_off - lo + ncall) * 8],
                                nidx, nidx_regs[nidx], NH,
                                queue_num=ci % cfg.NQUEUES)
                            if SKIP_SEG:
                                continue
                            cpos = 0
                            for t, nch in tl_list:
                                if (t, p) not in psum_of:
                                    pt = psg.tile(
                                        [P, NH], f32, space="PSUM",
                                        tag="ps", name=f"ps{t}p{p}")
                                    psum_of[(t, p)] = pt
                                    tile_seen[(t, p)] = 0
                                    if p == 1 and t in part_of:
                                        # fold phase-A partial into PSUM
                                        # via identity matmul
                                        nc.tensor.matmul(
                                            pt[:], lhsT=ib_t[:],
                                            rhs=part_of[t][:],
                                            start=True, stop=False,
                                            skip_group_check=True)
                                        part_merged.add(t)
                                pt = psum_of[(t, p)]
                                merged = p == 1 and t in part_merged
                                for j in range(nch):
                                    ch = ch_off - lo + cpos + j
                                    lane = (sctr * 37) % 100
                                    sctr += 1
                                    S_t = sbS.tile([P, P], hdt, tag="S")
                                    if lane < cfg.ACTS:
                                        a1 = sbS.tile([P, P], hdt, tag="a1")
                                        nc.scalar.activation(
                                            a1[:], iota_t, Act.Abs,
                                            bias=rneg_t[:, ch:ch + 1])
                                        nc.scalar.activation(
                                            S_t[:], a1[:], Act.Relu,
                                            bias=vals_t[:, ch:ch + 1],
                                            scale=vneg_t[:, ch:ch + 1])
                                    elif lane < cfg.ACTS + cfg.POOLS:
                                        nc.gpsimd.tensor_scalar(
                                            out=S_t[:], in0=iota_t,
                                            scalar1=rows_t[:, ch:ch + 1],
                                            scalar2=vals_t[:, ch:ch + 1],
                                            op0=Alu.is_equal, op1=Alu.mult)
                                    else:
                                        nc.vector.tensor_scalar(
                                            out=S_t[:], in0=iota_t,
                                            scalar1=rows_t[:, ch:ch + 1],
                                            scalar2=vals_t[:, ch:ch + 1],
                                            op0=Alu.is_equal, op1=Alu.mult)
                                    nc.tensor.matmul(
                                        pt[:],
                                        lhsT=S_t[:],
                                        rhs=G[:, (cpos + j) * NH:(cpos + j + 1) * NH],
                                        start=(tile_seen[(t, p)] == 0
                                               and not merged),
                                        stop=(tile_seen[(t, p)]
                                              == tile_total[(t, p)] - 1),
                                        skip_group_check=True)
                                    tile_seen[(t, p)] += 1
                                cpos += nch

                        if cfg.DEFER:
                            if pending is not None:
                                finish_group(p, *pending)
                            pending = (gi, g)
                        else:
                            finish_group(p, gi, g)
                    if pending is not None:
                        finish_group(p, *pending)
                if l < L - 1:
                    issue_ag(l + 1, 1)
    return nc


def _prepare(x, rows, cols, vals, W1, b1, W2, b2, W3, b3,
             ln_g, ln_b, Wout, bout, cfg):
    import os as _os
    x = np.ascontiguousarray(np.asarray(x, dtype=np.float32))
    rows = np.asarray(rows, dtype=np.int64)
    cols = np.asarray(cols, dtype=np.int64)
    vals = np.asarray(vals, dtype=np.float32)
    W1 = np.asarray(W1, dtype=np.float32)
    if _os.environ.get("GCN_BAL", "1") == "1":
        pi = _balance_perm(cfg, rows, cols)
        inv = np.empty_like(pi)
        inv[pi] = np.arange(cfg.N, dtype=np.int64)
        x = np.ascontiguousarray(x[pi])
        rows = inv[rows]
        cols = inv[cols]
        cfg.perm = pi
    else:
        cfg.perm = np.arange(cfg.N, dtype=np.int64)
    plan = _Plan(cfg, rows, cols)
    xT, W1p, idx_arr, row_arr, val_arr = _preprocess(
        cfg, plan, x, rows, cols, vals, W1)

    rep = np.ones((P, 1), np.float32)
    b_np = [np.asarray(b, np.float32) for b in (b1, b2, b3)]
    apply_b = [not np.all(b == 0) for b in b_np]
    ln_g = np.asarray(ln_g, np.float32)
    ln_b = np.asarray(ln_b, np.float32)
    bout = np.asarray(bout, np.float32)
    apply_g = not np.all(ln_g == 1)
    apply_lnb = not np.all(ln_b == 0)
    apply_bout = not np.all(bout == 0)

    nc = _build_program(cfg, plan, apply_b, apply_g, apply_lnb, apply_bout)
    nc.compile()

    if cfg.BF16:
        import concourse.mybir as mybir
        bf = mybir.dt.np(mybir.dt.bfloat16)
        xT = xT.astype(bf)
        W1p = W1p.astype(bf)

    in_maps = []
    for c in range(cfg.CORES):
        consts = np.concatenate(
            [np.tile(np.arange(P, dtype=np.float32)[None, :], (P, 1)),
             np.eye(P, dtype=np.float32)], axis=1)
        m = {
            "xT": xT[c],
            "W1p": W1p,
            "consts": np.ascontiguousarray(consts),
            "W2": np.asarray(W2, np.float32),
            "W3": np.asarray(W3, np.float32),
            "Wout": np.asarray(Wout, np.float32),
            "bout": np.ascontiguousarray(rep * bout[None, :]),
            "lng": np.ascontiguousarray(rep * ln_g[None, :]),
            "lnb": np.ascontiguousarray(rep * ln_b[None, :]),
        }
        import concourse.mybir as _mybir
        m["identb"] = np.eye(P, dtype=_mybir.dt.np(_mybir.dt.bfloat16)
                             if cfg.BF16 else np.float32)
        for l in range(3):
            m[f"b{l+1}"] = np.ascontiguousarray(rep * b_np[l][None, :])
            m[f"idx{l}"] = idx_arr[c][l]
            # pack rows|vals|rneg|vneg per (phase, group) span
            meta = np.zeros((P, plan.tot_ch[l] * 4), np.float32)
            rr, vv_ = row_arr[c][l], val_arr[c][l]
            for (lo, hi, _cs) in plan.group_spans[l].values():
                w = hi - lo
                if w <= 0:
                    continue
                b0 = lo * 4
                meta[:, b0:b0 + w] = rr[:, lo:hi]
                meta[:, b0 + w:b0 + 2 * w] = vv_[:, lo:hi]
                meta[:, b0 + 2 * w:b0 + 3 * w] = -rr[:, lo:hi]
                meta[:, b0 + 3 * w:b0 + 4 * w] = -vv_[:, lo:hi]
            m[f"meta{l}"] = meta
        in_maps.append(m)

    return nc, in_maps


def kernel(**inputs):
    from concourse.bass_utils import run_bass_kernel_spmd
    cfg = _Cfg(N=100000, NFEAT=602, NHID=128, NCLASS=41, CORES=8)
    nc, in_maps = _prepare(cfg=cfg, **inputs)
    res = run_bass_kernel_spmd(nc, in_maps, core_ids=list(range(cfg.CORES)))
    out = np.concatenate([r["out"] for r in res.results], axis=0)
    fin = np.empty_like(out)
    fin[cfg.perm] = out
    return np.ascontiguousarray(fin)
